# revision 29
# baseline (speedup 1.0000x reference)
"""Bass/Trainium2 kernel for a fused GRU cell.

  r   = sigmoid(x @ W_ir.T + h @ W_hr.T + b_r)
  z   = sigmoid(x @ W_iz.T + h @ W_hz.T + b_z)
  g   = tanh  (x @ W_ih.T + (r*h) @ W_hh.T + b_h)
  h_t = (1-z)*h + z*g

Sharding: data-parallel over the batch (8192 -> 1024 rows per core on 8
NeuronCores), weights replicated, no collectives.

On-device layout is transposed ([hidden, batch], hidden on SBUF
partitions) so per-h-tile biases are per-partition scalars and all DMAs
are contiguous.  Matmul operands (weights, x, h, r*h) are bf16: the PE
runs bf16 at the same 1 col/cycle as fp32r but the stationary load is a
separate LDWEIGHTS the PE can hoist, DMA bytes halve, and fp32
accumulation in PSUM keeps the error ~1e-3.  A second fp32 copy of
h_prev is kept for the h_t = h + z*(g-h) epilogue.

DMA issue order == consumption order (first weight slabs before the
bulk activation loads) so the first matmul starts ~1 us in instead of
waiting for every activation byte.
"""

import sys

for _p in ("/opt/trn_rl_repo", "/root/.axon_site/_ro/trn_rl_repo"):
    if _p not in sys.path:
        sys.path.append(_p)

import numpy as np

P = 128          # SBUF partitions
BC_MAX = 512     # PSUM-bank max free dim (fp32 accumulate)
N_CORES = 8
QT = 8           # weight k-tiles per DMA slab (2KB/partition in bf16)

_PROG_CACHE = {}


def build_program(Bc, IN, H):
    """Build the per-core SPMD Bass program (identical on all cores)."""
    from contextlib import ExitStack

    from concourse import bacc, bass, mybir, tile
    from concourse.dt import dt

    KI, KH, NT = IN // P, H // P, H // P
    NJ = KI + KH                 # contraction tiles per gate per h-tile
    NQ = NJ // QT
    assert KI == QT and NJ % QT == 0  # head scheduling assumes slab q=0 == x-part
    BC = min(BC_MAX, Bc)
    NB = Bc // BC
    f32, bf16 = dt.float32, dt.bfloat16
    SIG = mybir.ActivationFunctionType.Sigmoid
    TANH = mybir.ActivationFunctionType.Tanh

    nc = bacc.Bacc("TRN2", debug=False)
    xt_d = nc.declare_dram_parameter("xt", [P, KI, Bc], bf16, False)
    hpm_d = nc.declare_dram_parameter("hpm", [P, KH, Bc], bf16, False)
    hp32_d = nc.declare_dram_parameter("hp32", [P, KH, Bc], f32, False)
    wr_d = nc.declare_dram_parameter("wr", [NT, NQ, P, QT * P], bf16, False)
    wz_d = nc.declare_dram_parameter("wz", [NT, NQ, P, QT * P], bf16, False)
    wh_d = nc.declare_dram_parameter("wh", [NT, NQ, P, QT * P], bf16, False)
    b_d = nc.declare_dram_parameter("bias", [P, NT * 3], f32, False)
    out_d = nc.declare_dram_parameter("out", [NT, P, Bc], f32, True)

    with ExitStack() as ctx:
        tc = ctx.enter_context(tile.TileContext(nc))
        res = ctx.enter_context(tc.tile_pool(name="res", bufs=1))
        wp = ctx.enter_context(tc.tile_pool(name="wp", bufs=15))
        pp = ctx.enter_context(
            tc.tile_pool(name="pp", bufs=3, space=bass.MemorySpace.PSUM)
        )
        pq = ctx.enter_context(
            tc.tile_pool(name="pq", bufs=1, space=bass.MemorySpace.PSUM)
        )
        op = ctx.enter_context(tc.tile_pool(name="op", bufs=4))
        zp = ctx.enter_context(tc.tile_pool(name="zp", bufs=3))

        xt = res.tile([P, KI, Bc], bf16, tag="xt")
        hpm = res.tile([P, KH, Bc], bf16, tag="hpm")
        hp32 = res.tile([P, KH, Bc], f32, tag="hp32")
        rh = res.tile([P, KH, Bc], bf16, tag="rh")
        bias = res.tile([P, NT * 3], f32, tag="bias")
        wu = res.tile([P, BC], bf16, tag="wu")

        def load_slab(w_d, hti, q):
            slab = wp.tile([P, QT * P], bf16, tag="w")
            nc.sync.dma_start(out=slab[:], in_=w_d[hti, q])
            return slab

        # -------- HAM warm-up --------
        # The PE sits DMA-bound for the first ~10us; run throwaway
        # matmuls on zeroed scratch so the activity monitor lifts the
        # clock gate (1.2 -> 2.4 GHz) before the real stream begins.
        nc.gpsimd.memset(wu[:], 0.0)
        wups = pp.tile([P, Bc], f32, tag="ps")
        for _ in range(8):
            nc.tensor.matmul(
                wups[:, 0:BC], wu[:, 0:P], wu[:, 0:BC],
                start=True, stop=True, skip_group_check=True,
            )

        # -------- DMA issue order == consumption order --------
        # The 16 rings service bytes globally in issue order, so anything
        # issued ahead of a weight slab delays it.  The first NTI r-gate
        # tiles run x-parts first (xt arrives early) with their h-parts
        # j-major against the streaming hpm tiles; issue their slabs
        # interleaved with the act loads.  hp32 (epilogue only) streams
        # later, one tile per loop iteration.  The very first slab/xt
        # tiles are split into pieces so their consumers' DMA semaphores
        # fire as early as possible.
        NTI = 3
        s00 = wp.tile([P, QT * P], bf16, tag="w")
        nc.sync.dma_start(out=s00[:, 0:P], in_=wr_d[0, 0, :, 0:P])
        nc.sync.dma_start(out=xt[:, 0, 0:BC], in_=xt_d[:, 0, 0:BC])
        nc.sync.dma_start(out=xt[:, 0, BC:Bc], in_=xt_d[:, 0, BC:Bc])
        nc.sync.dma_start(out=s00[:, P : 4 * P], in_=wr_d[0, 0, :, P : 4 * P])
        nc.sync.dma_start(out=s00[:, 4 * P : QT * P], in_=wr_d[0, 0, :, 4 * P : QT * P])
        pre = {(0, 0): s00}
        nc.sync.dma_start(out=bias[:], in_=b_d[:])
        pre[(1, 0)] = load_slab(wr_d, 1, 0)
        for j in range(1, 4):
            nc.sync.dma_start(out=xt[:, j, :], in_=xt_d[:, j, :])
        pre[(2, 0)] = load_slab(wr_d, 2, 0)
        for j in range(4, KI):
            nc.sync.dma_start(out=xt[:, j, :], in_=xt_d[:, j, :])
        for t in (0, 1):
            for q in (1, 2):
                pre[(t, q)] = load_slab(wr_d, t, q)
        # tile 3's slabs by bytes ahead of the hpm bulk: issued here they
        # arrive ~28us, well before tile 3's matmuls (~46us); left in the
        # R loop they queue behind 8.7MB and arrive ~47us (914ns PE gap)
        for q in range(NQ):
            pre[(3, q)] = load_slab(wr_d, 3, q)
        for t in range(KH):
            nc.sync.dma_start(out=hpm[:, t, :], in_=hpm_d[:, t, :])
            if t < 2:
                pre[(2, 1 + t)] = load_slab(wr_d, 2, 1 + t)

        def gate(ps, w_d, hti, srch, preload=None, bc_outer=False):
            # ps[:, bc] += sum_j W_tile[j].T @ moving[j][:, bc]
            slabs = []
            for q in range(NQ):
                if preload is not None and (hti, q) in preload:
                    slabs.append(preload[(hti, q)])
                else:
                    slabs.append(load_slab(w_d, hti, q))

            def mms(bcs):
                for q in range(NQ):
                    for jj in range(QT):
                        j = q * QT + jj
                        lhs = slabs[q][:, jj * P : (jj + 1) * P]
                        mov = xt[:, j, :] if j < KI else srch[:, j - KI, :]
                        for bc in bcs:
                            nc.tensor.matmul(
                                ps[:, bc * BC : (bc + 1) * BC],
                                lhs,
                                mov[:, bc * BC : (bc + 1) * BC],
                                start=(j == 0),
                                stop=(j == NJ - 1),
                                skip_group_check=True,
                            )

            if bc_outer:
                for bc in range(NB):
                    mms([bc])
            else:
                mms(range(NB))

        # ---- phase R: r = sigmoid(gi_r + gh_r + b_r); rh = r * h ----
        def r_epilogue(hti, ps):
            for bc in range(NB):
                sl = slice(bc * BC, (bc + 1) * BC)
                rs = zp.tile([P, BC], f32, tag="zs")
                nc.scalar.activation(
                    rs[:], ps[:, sl], SIG, bias=bias[:, hti * 3 : hti * 3 + 1]
                )
                nc.vector.tensor_mul(rh[:, hti, sl], rs[:], hpm[:, hti, sl])

        # interleaved head: x-parts of tiles 0..NTI-1 (only need xt),
        # then their h-parts j-major so consumption tracks the hpm stream
        head_ps = []
        for t in range(NTI):
            hps = pp.tile([P, Bc], f32, tag="ps")
            head_ps.append(hps)
        for t in range(NTI):
            slab = pre[(t, 0)]
            for jj in range(QT):
                lhs = slab[:, jj * P : (jj + 1) * P]
                for bc in range(NB):
                    nc.tensor.matmul(
                        head_ps[t][:, bc * BC : (bc + 1) * BC],
                        lhs,
                        xt[:, jj, bc * BC : (bc + 1) * BC],
                        start=(jj == 0),
                        stop=False,
                        skip_group_check=True,
                    )
        for j in range(KI, NJ):
            q, jj = j // QT, j % QT
            for t in range(NTI):
                lhs = pre[(t, q)][:, jj * P : (jj + 1) * P]
                for bc in range(NB):
                    nc.tensor.matmul(
                        head_ps[t][:, bc * BC : (bc + 1) * BC],
                        lhs,
                        hpm[:, j - KI, bc * BC : (bc + 1) * BC],
                        start=False,
                        stop=(j == NJ - 1),
                        skip_group_check=True,
                    )
        for t in range(NTI):
            r_epilogue(t, head_ps[t])

        for hti in range(NTI, NT):
            nc.sync.dma_start(out=hp32[:, hti - NTI, :], in_=hp32_d[:, hti - NTI, :])
            ps = pp.tile([P, Bc], f32, tag="ps")
            gate(ps, wr_d, hti, hpm, preload=pre if hti == NTI else None)
            r_epilogue(hti, ps)

        # ---- phase ZH: z, g, h_t = h + z*(g - h) ----
        def zh_epilogue(hti, psz, psh, sl, ec):
            zs = zp.tile([P, BC], f32, tag="zs")
            for e in range(ec):
                w = BC // ec
                s = slice(sl.start + e * w, sl.start + (e + 1) * w)
                so = slice(e * w, (e + 1) * w)
                nc.scalar.activation(
                    zs[:, so], psz[:, s], SIG, bias=bias[:, hti * 3 + 1 : hti * 3 + 2]
                )
                nc.scalar.activation(
                    psh[:, s], psh[:, s], TANH, bias=bias[:, hti * 3 + 2 : hti * 3 + 3]
                )
                # h_t = h + z*(g - h); DVE reads at most ONE psum operand/op
                nc.vector.tensor_sub(psh[:, s], psh[:, s], hp32[:, hti, s])
                nc.vector.tensor_mul(psh[:, s], zs[:, so], psh[:, s])
                o = op.tile([P, BC // ec], f32, tag="o")
                nc.vector.tensor_add(o[:], psh[:, s], hp32[:, hti, s])
                nc.gpsimd.dma_start(out=out_d[hti, :, s], in_=o[:])

        for hti in range(NT):
            last = hti == NT - 1
            if hti < NTI:
                nc.sync.dma_start(
                    out=hp32[:, NT - NTI + hti, :], in_=hp32_d[:, NT - NTI + hti, :]
                )
            psz = pp.tile([P, Bc], f32, tag="ps")
            gate(psz, wz_d, hti, hpm)
            if not last:
                psh = pp.tile([P, Bc], f32, tag="ps")
                gate(psh, wh_d, hti, rh)
                for bc in range(NB):
                    zh_epilogue(hti, psz, psh, slice(bc * BC, (bc + 1) * BC), 1)
                continue
            # Final tile: keep the post-last-matmul critical path minimal.
            # z and t1 = (1-z)*h only need psz, so they run during the
            # h-gate matmuls; afterwards just TANH -> z*g -> +t1 -> DMA.
            # The h-gate runs as three column groups (512, 256, 256) so
            # each group's epilogue hides under the next group's matmuls.
            zs = zp.tile([P, Bc], f32, tag="zs")
            t1 = zp.tile([P, Bc], f32, tag="zs")
            for bc in range(NB):
                sl = slice(bc * BC, (bc + 1) * BC)
                nc.scalar.activation(
                    zs[:, sl], psz[:, sl], SIG,
                    bias=bias[:, hti * 3 + 1 : hti * 3 + 2],
                )
                nc.vector.tensor_mul(t1[:, sl], zs[:, sl], hp32[:, hti, sl])
                nc.vector.tensor_sub(t1[:, sl], hp32[:, hti, sl], t1[:, sl])

            lslabs = [load_slab(wh_d, hti, q) for q in range(NQ)]

            def h_group(ps, cols):
                for q in range(NQ):
                    for jj in range(QT):
                        j = q * QT + jj
                        lhs = lslabs[q][:, jj * P : (jj + 1) * P]
                        mov = xt[:, j, cols] if j < KI else rh[:, j - KI, cols]
                        nc.tensor.matmul(
                            ps[:, 0 : cols.stop - cols.start], lhs, mov,
                            start=(j == 0), stop=(j == NJ - 1),
                            skip_group_check=True,
                        )

            def h_epilogue(ps, cols):
                w = cols.stop - cols.start
                nc.scalar.activation(
                    ps[:, 0:w], ps[:, 0:w], TANH,
                    bias=bias[:, hti * 3 + 2 : hti * 3 + 3],
                )
                nc.vector.tensor_mul(ps[:, 0:w], zs[:, cols], ps[:, 0:w])
                o = op.tile([P, 512], f32, tag="o")
                nc.vector.tensor_add(o[:, 0:w], ps[:, 0:w], t1[:, cols])
                nc.gpsimd.dma_start(out=out_d[hti, :, cols], in_=o[:, 0:w])

            # Three column groups in three separate PSUM tiles (so their
            # epilogues never WAR-block the next group's matmuls): each
            # group's epilogue hides under the following group's stream.
            psh_a = pp.tile([P, Bc], f32, tag="ps")
            psh_b1 = pq.tile([P, 256], f32, tag="pb1")
            psh_b2 = pq.tile([P, 256], f32, tag="pb2")
            ga, gb1, gb2 = slice(0, BC), slice(BC, BC + 256), slice(BC + 256, Bc)
            h_group(psh_a, ga)
            h_group(psh_b1, gb1)
            h_epilogue(psh_a, ga)
            h_group(psh_b2, gb2)
            h_epilogue(psh_b1, gb1)
            h_epilogue(psh_b2, gb2)

    nc.compile()
    return nc


def _pack_weight_gate(Wi, Wh, dtype):
    """Stack [Wi-tiles; Wh-tiles] -> (NT, NQ, P, QT*P) DMA-slab layout.

    slab[hti, q][p, jj*P + m] = W[hti*P + m, k] with k = (q*QT+jj tile)*P + p,
    i.e. each 128x128 stationary tile is W.T for that (k-tile, h-tile) block.
    """
    H, IN = Wi.shape
    KI, KH, NT = IN // P, H // P, H // P
    ti = Wi.reshape(NT, P, KI, P).transpose(0, 2, 3, 1)  # (NT, KI, p, m)
    th = Wh.reshape(NT, P, KH, P).transpose(0, 2, 3, 1)  # (NT, KH, p, m)
    cat = np.concatenate([ti, th], axis=1)               # (NT, NJ, p, m)
    NJ = KI + KH
    NQ = NJ // QT
    return np.ascontiguousarray(
        cat.reshape(NT, NQ, QT, P, P).transpose(0, 1, 3, 2, 4)
        .reshape(NT, NQ, P, QT * P)
    ).astype(dtype)


def _pack_acts(a, dtype):
    """(Bc, D) -> (P, D//P, Bc) with [p, t, b] = a[b, t*P + p]."""
    Bc, D = a.shape
    return np.ascontiguousarray(
        a.T.reshape(D // P, P, Bc).transpose(1, 0, 2)
    ).astype(dtype)


def run(x_t, h_prev, W_ir, W_iz, W_ih, W_hr, W_hz, W_hh, b_r, b_z, b_h,
        trace=False):
    import ml_dtypes
    from concourse.bass_utils import run_bass_kernel_spmd

    bf16 = ml_dtypes.bfloat16
    x_t = np.asarray(x_t, dtype=np.float32)
    h_prev = np.asarray(h_prev, dtype=np.float32)
    B, IN = x_t.shape
    H = h_prev.shape[1]
    assert B % N_CORES == 0
    Bc = B // N_CORES
    NT = H // P

    key = (Bc, IN, H)
    if key not in _PROG_CACHE:
        _PROG_CACHE[key] = build_program(Bc, IN, H)
    nc = _PROG_CACHE[key]

    wr = _pack_weight_gate(np.asarray(W_ir, np.float32), np.asarray(W_hr, np.float32), bf16)
    wz = _pack_weight_gate(np.asarray(W_iz, np.float32), np.asarray(W_hz, np.float32), bf16)
    wh = _pack_weight_gate(np.asarray(W_ih, np.float32), np.asarray(W_hh, np.float32), bf16)
    bias = np.ascontiguousarray(
        np.stack(
            [np.asarray(b_r, np.float32), np.asarray(b_z, np.float32),
             np.asarray(b_h, np.float32)], axis=-1
        ).reshape(NT, P, 3).transpose(1, 0, 2).reshape(P, NT * 3)
    )

    in_maps = []
    for c in range(N_CORES):
        rows = slice(c * Bc, (c + 1) * Bc)
        hp_packed = _pack_acts(h_prev[rows], np.float32)
        in_maps.append({
            "xt": _pack_acts(x_t[rows], bf16),
            "hpm": hp_packed.astype(bf16),
            "hp32": hp_packed,
            "wr": wr, "wz": wz, "wh": wh, "bias": bias,
        })

    kw = {}
    if trace:
        kw = dict(trace=True, trace_cores=[0])
    res = run_bass_kernel_spmd(nc, in_maps, core_ids=list(range(N_CORES)), **kw)

    outs = []
    for c in range(N_CORES):
        o = res.results[c]["out"]          # (NT, P, Bc)
        outs.append(o.reshape(H, Bc).T)    # (Bc, H)
    full = np.concatenate(outs, axis=0).astype(np.float32)
    return (full, res) if trace else full


def kernel(**inputs):
    return run(**inputs)
